# revision 56
# baseline (speedup 1.0000x reference)
"""Position-attention kernel for Trainium2 (8 NeuronCores, Bass/Tile).

Module: q,k = 1x1 convs to C/8 channels, v = 1x1 conv, attn = softmax(q^T k),
y = v @ attn^T, out = gamma*y + x.  Shapes: B=4, C=512, H=W=64 (N=4096, Cq=64).

Sharding: data-parallel over batch x query-halves -> 8 cores. Core i handles
batch i//2, query positions [h*2048, (h+1)*2048) with h = i%2.

v2 design (all-fp8 projections, wide exps, detached epilogue):
- x ships once as fp8e4 in DoubleRow pair layout; K/Q/V projections all run as
  fp8 DR matmuls in the same (128,128) PE tiling mode as the attention loop, so
  the PE never mode-switches. q/k weights are scaled x16 (v: x32*gamma) on host
  to clear the fp8e4 denormal range; the PSUM-drain activations descale.
- K/Q weights are zero-padded to 128 output columns, so the k/q SBUF tiles come
  out zero-padded to 128 partitions with no memsets.
- Main loop per t (2 key tiles): 2 bf16 ST matmuls into a 2-bank PSUM pair, ONE
  1024-wide exp -> fp8e5, a bf16 running denominator add on DVE, 4 fp8-DR U
  matmuls. U matmuls are emitted one t behind the STs to keep the PE queue from
  stalling on ACT.
- Per-chunk epilogue: u is drained PSUM->SBUF as bf16 on DVE (frees the PSUM
  banks fast), denominator partition-reduce on GpSimd (idle engine, hidden
  latency), reciprocal + normalize + residual on DVE, bf16 output DMA. The last
  chunk reduces via an all-ones PE matmul instead to shorten the tail.
- Residual ships as bf16 with gamma*v_b pre-added; output DMA is bf16 and the
  host upcasts.
"""

import numpy as np
import ml_dtypes

import concourse.bass as bass
import concourse.mybir as mybir
import concourse.tile as tile
from concourse import bacc, bass_isa
from concourse.bass_utils import run_bass_kernel_spmd

BF16 = ml_dtypes.bfloat16
F8E4NP = ml_dtypes.float8_e4m3

B, C, H, W = 4, 512, 64, 64
N = H * W            # 4096 keys per batch
NQ = N // 2          # 2048 queries per core
CQ = C // 8          # 64 q/k channels
P = 128
CT = C // P          # 4 channel tiles
TP = 2               # channel-tile pairs (DoubleRow)
MT = N // P          # 32 key tiles
NCH = 512            # matmul moving-dim chunk
QCH = NQ // NCH      # 4 query chunks per core
KCH = N // NCH       # 8 key chunks
NCORES = 8

F32 = mybir.dt.float32
BF = mybir.dt.bfloat16
F8 = mybir.dt.float8e4
F8E = mybir.dt.float8e5
AF = mybir.ActivationFunctionType
DR = mybir.MatmulPerfMode.DoubleRow
LN16 = 2.772588722239781  # exp shift (ln 16): max logit ~10.9 -> e^8.1 < fp8e5 max
SQ = 16.0            # host scale on q/k weights (fp8e4 denormal avoidance)
SV = 32.0            # host scale on gamma*v weights

_CACHE = {}


def _build_program():
    nc = bacc.Bacc()

    x8 = nc.declare_dram_parameter("x8", [P, TP * TP * N], F8, isOutput=False)
    xr = nc.declare_dram_parameter("xr", [C, NQ], BF, isOutput=False)
    qw = nc.declare_dram_parameter("qw", [P, TP * TP * P], F8, isOutput=False)
    kw = nc.declare_dram_parameter("kw", [P, TP * TP * P], F8, isOutput=False)
    vw = nc.declare_dram_parameter("vw", [P, TP * TP * C], F8, isOutput=False)
    qb = nc.declare_dram_parameter("qb", [P, 1], F32, isOutput=False)
    kb = nc.declare_dram_parameter("kb", [P, 1], F32, isOutput=False)
    out = nc.declare_dram_parameter("out", [C, NQ], BF, isOutput=True)

    with tile.TileContext(nc) as tc:
        with tc.tile_pool(name="consts", bufs=1) as consts:
            x8_sb = consts.tile([P, TP * TP * N], F8)
            qw_sb = consts.tile([P, TP * TP * P], F8)
            kw_sb = consts.tile([P, TP * TP * P], F8)
            vw_sb = consts.tile([P, TP * TP * C], F8)
            qb_sb = consts.tile([P, 1], F32)
            kb_sb = consts.tile([P, 1], F32)
            xr_sb = consts.tile([P, CT * NQ], BF)
            k_sb = consts.tile([P, N], BF)
            q_sb = consts.tile([P, NQ], BF)
            vt_sb = consts.tile([P, MT * C], F8)       # vT: 32 m-tiles of [128, 512]
            ln16_sb = consts.tile([P, 1], F32)
            ones_sb = consts.tile([P, P], BF)
            ones8_sb = consts.tile([P, P], F8)

            # ---- input DMAs ----
            # first x8 slab + small weights first so the K projection can
            # start as early as possible; residual last (not needed until the
            # first epilogue).
            x8_r = x8[:, :].rearrange("p (t n) -> p t n", t=TP * TP)
            x8sb_r = x8_sb.rearrange("p (t n) -> p t n", t=TP * TP)

            def x8_slab(q, s):
                q.dma_start(out=x8sb_r[:, :, s * NCH:(s + 1) * NCH],
                            in_=x8_r[:, :, s * NCH:(s + 1) * NCH])

            nc.scalar.dma_start(out=kw_sb, in_=kw[:, :])
            x8_slab(nc.sync, 0)
            nc.scalar.dma_start(out=qw_sb, in_=qw[:, :])
            nc.scalar.dma_start(out=kb_sb, in_=kb[:, :])
            nc.scalar.dma_start(out=qb_sb, in_=qb[:, :])
            nc.sync.dma_start(out=vw_sb, in_=vw[:, :])
            for s in range(1, KCH):
                x8_slab(nc.sync if s % 2 == 0 else nc.scalar, s)
            nc.scalar.dma_start(out=xr_sb.rearrange("p (t m) -> p t m", t=CT),
                                in_=xr[:, :].rearrange("(t p) m -> p t m", p=P))

            nc.vector.memset(ln16_sb, -LN16)
            nc.vector.memset(ones_sb, 1.0)
            nc.vector.memset(ones8_sb, 1.0)
            # Touch bias tiles on ACT (single sync-wait slot on the
            # activation-with-bias struct) + preload the Exp table set early.
            bias_touch = consts.tile([P, 2], F32)
            nc.scalar.activation(bias_touch[:, 0:1], kb_sb, AF.Copy)
            nc.scalar.activation(bias_touch[:, 1:2], qb_sb, AF.Copy)
            exp_touch = consts.tile([P, 1], F32)
            nc.scalar.activation(exp_touch, ln16_sb, AF.Exp, bias=ln16_sb)

            kw_r = kw_sb.rearrange("p (t j o) -> p t j o", t=TP, j=TP)
            qw_r = qw_sb.rearrange("p (t j o) -> p t j o", t=TP, j=TP)
            vw_r = vw_sb.rearrange("p (t j o) -> p t j o", t=TP, j=TP)
            x8_4 = x8_sb.rearrange("p (t j n) -> p t j n", t=TP, j=TP)

            # ---- K/Q projections (all fp8 DoubleRow) ----
            # V projections are NOT emitted here: they are interleaved into
            # chunk 0 of the attention loop below (their PSUM quads share the
            # st pool; quad t produces exactly the vt pair iteration t needs),
            # so their PE work fills dependency-wait slack and their descale
            # copies hide under the exps.
            with tc.tile_pool(name="proj_kq", bufs=2, space="PSUM") as proj_kq:
                def kq_drain(dst, src, bias, alt):
                    # descale + bias: alternate ACT / DVE so neither engine's
                    # PSUM-ring drain gates the projection matmul stream
                    if alt:
                        nc.vector.tensor_scalar(dst, src, 1.0 / SQ, bias,
                                                mybir.AluOpType.mult,
                                                mybir.AluOpType.add)
                    else:
                        nc.scalar.activation(dst, src, AF.Identity, bias=bias,
                                             scale=1.0 / SQ)

                def k_proj(ch):
                    kp = proj_kq.tile([P, NCH], F32, tag="kq", name="kp")
                    for tp in range(TP):
                        nc.tensor.matmul(
                            kp, lhsT=kw_r[:, tp, :, :],
                            rhs=x8_4[:, tp, :, ch * NCH:(ch + 1) * NCH],
                            start=(tp == 0), stop=(tp == TP - 1), perf_mode=DR)
                    kq_drain(k_sb[:, ch * NCH:(ch + 1) * NCH], kp, kb_sb,
                             ch % 2 == 1)

                def q_proj(ch):
                    qp = proj_kq.tile([P, NCH], F32, tag="kq", name="qp")
                    for tp in range(TP):
                        nc.tensor.matmul(
                            qp, lhsT=qw_r[:, tp, :, :],
                            rhs=x8_4[:, tp, :, ch * NCH:(ch + 1) * NCH],
                            start=(tp == 0), stop=(tp == TP - 1), perf_mode=DR)
                    kq_drain(q_sb[:, ch * NCH:(ch + 1) * NCH], qp, qb_sb,
                             ch % 2 == 0)

                for s in range(KCH):
                    k_proj(s)
                    if s < QCH:
                        q_proj(s)

            # ---- attention main loop ----
            vt_r = vt_sb.rearrange("p (m c) -> p m c", m=MT)
            out_r = out[:, :].rearrange("(c p) n -> p c n", p=P)
            xr_r = xr_sb.rearrange("p (c m) -> p c m", c=CT)
            with (
                tc.tile_pool(name="u_ps", bufs=1, space="PSUM") as u_ps,
                tc.tile_pool(name="st_ps", bufs=2, space="PSUM") as st_ps,
                tc.tile_pool(name="e_pool", bufs=6) as e_pool,
                tc.tile_pool(name="cs_pool", bufs=2) as cs_pool,
                tc.tile_pool(name="fin", bufs=2) as fin,
                tc.tile_pool(name="outp", bufs=2) as outp,
            ):
                def v_proj(mt2):
                    # two key tiles (2*mt2, 2*mt2+1) -> one [128, 1024] PSUM
                    # quad (borrowed from the st ring) -> one wide descaling
                    # copy to fp8 vt.
                    vp = st_ps.tile([P, 2 * NCH], F32, tag="st", name="vp")
                    for half in range(2):
                        mt = 2 * mt2 + half
                        for tp in range(TP):
                            nc.tensor.matmul(
                                vp[:, half * NCH:(half + 1) * NCH],
                                lhsT=x8_4[:, tp, :, mt * P:(mt + 1) * P],
                                rhs=vw_r[:, tp, :, :],
                                start=(tp == 0), stop=(tp == TP - 1),
                                perf_mode=DR)
                    dst = vt_sb[:, 2 * mt2 * C:(2 * mt2 + 2) * C]
                    # alternate the f32->fp8 descale copies between ACT and
                    # DVE; DVE takes the EVEN quads so its copy never lands on
                    # the same iteration as the cs pair-add (odd t)
                    if mt2 % 2 == 1:
                        nc.scalar.activation(dst, vp, AF.Copy, scale=1.0 / SV)
                    else:
                        nc.vector.tensor_scalar_mul(dst, vp, 1.0 / SV)

                for ch in range(QCH):
                    last = ch == QCH - 1
                    u = u_ps.tile([P, CT * NCH], F32, tag="u", name="u")
                    cs_d = cs_pool.tile([P, 4 * NCH], BF, tag="csd", name="cs_d")
                    cs_n = cs_d[:, :2 * NCH]   # last chunk: fine-grained accum
                    qs = q_sb[:, ch * NCH:(ch + 1) * NCH]

                    pend = []   # deferred U matmuls (one t behind)

                    def flush_u():
                        while pend:
                            pend.pop(0)()

                    ep = None
                    for t in range(MT // 2):
                        if ch == 0:
                            # interleave the V projection: quad t produces the
                            # vt pair that this iteration's (deferred) U uses
                            v_proj(t)
                        stp = st_ps.tile([P, 2 * NCH], F32, tag="st", name="st")
                        for j in range(2):
                            nc.tensor.matmul(
                                stp[:, j * NCH:(j + 1) * NCH],
                                lhsT=k_sb[:, (2 * t + j) * P:(2 * t + j + 1) * P],
                                rhs=qs, start=True, stop=True)
                        if t % 2 == 0:
                            ep = e_pool.tile([P, 2, 2 * NCH], F8E, tag="e", name="ep")
                        half = t % 2
                        e1k = ep[:, half, :]
                        nc.scalar.activation(e1k, stp, AF.Exp, bias=ln16_sb)
                        flush_u()

                        def emit_u(t=t, e2=e1k.rearrange("p (j n) -> p j n", j=2)):
                            for c in range(CT):
                                nc.tensor.matmul(
                                    u[:, c * NCH:(c + 1) * NCH],
                                    lhsT=vt_r[:, 2 * t:2 * t + 2, c * P:(c + 1) * P],
                                    rhs=e2,
                                    start=(t == 0), stop=(t == MT // 2 - 1),
                                    perf_mode=DR)
                        pend.append(emit_u)

                        # the final pair (t=14,15) is never accumulated on
                        # DVE: the epilogue's all-ones PE matmuls read its
                        # exp tile directly in fp8 (keeps DVE under the
                        # per-chunk wall so rec never drifts late)
                        if t >= MT // 2 - 2:
                            if t == MT // 2 - 1:
                                ep7 = ep
                        elif last:
                            # fine-grained per-t adds (tail latency)
                            if t == 0:
                                nc.vector.tensor_copy(cs_n, e1k)
                            else:
                                nc.vector.tensor_add(cs_n, cs_n, e1k)
                        elif half == 1:
                            # accumulate the finished pair tile (2048 wide)
                            pt = t // 2
                            epf = ep.rearrange("p j n -> p (j n)")
                            if pt == 0:
                                nc.vector.tensor_copy(cs_d, epf)
                            else:
                                nc.vector.tensor_add(cs_d, cs_d, epf)
                    flush_u()

                    # ---- epilogue ----
                    if not last:
                        # free the U PSUM banks quickly: bf16 drain on ACT
                        uc = outp.tile([P, CT * NCH], BF, tag="uc", name="uc")
                        nc.scalar.activation(uc[:, :2 * NCH], u[:, :2 * NCH], AF.Copy)
                        nc.scalar.activation(uc[:, 2 * NCH:], u[:, 2 * NCH:], AF.Copy)
                        u_src = uc
                    else:
                        u_src = u
                    # fold + partition-reduce + broadcast via accumulating
                    # all-ones PE matmuls. Non-last chunks write into the u
                    # c0 bank (just freed by the drain, so the st ring isn't
                    # stolen from the next chunk's STs); the last chunk uses
                    # a free st bank since its u is still being read.
                    # the reduce matmuls write into a free st bank: both st
                    # buffers are idle at chunk end, and with the cs chain
                    # finishing early the DVE reciprocal frees it quickly —
                    # unlike the u bank, this does not chain the next chunk's
                    # STs behind the ACT drain in PE's in-order queue
                    rec = fin.tile([P, NCH], F32, tag="rec", name="rec")
                    csp = st_ps.tile([P, 2 * NCH], F32, tag="st",
                                     name="csp")[:, :NCH]
                    if not last:
                        csrc, nblk = cs_d, 4
                    else:
                        csrc, nblk = cs_n, 2
                    for b in range(nblk):
                        nc.tensor.matmul(csp, lhsT=ones_sb,
                                         rhs=csrc[:, b * NCH:(b + 1) * NCH],
                                         start=(b == 0), stop=False)
                    ep7f = ep7.rearrange("p h n -> p (h n)")
                    for b in range(4):
                        nc.tensor.matmul(csp, lhsT=ones8_sb,
                                         rhs=ep7f[:, b * NCH:(b + 1) * NCH],
                                         start=False, stop=(b == 3))
                    nc.vector.reciprocal_approx_fast(out=rec, in_=csp)
                    if not last:
                        # bf16 rec (on ACT) lets the normalize muls hit DVE's
                        # 2x 16-bit mode
                        rec_h = fin.tile([P, NCH], BF, tag="rech", name="rec_h")
                        nc.scalar.activation(rec_h, rec, AF.Copy)
                    else:
                        # tail: skip the cross-engine cast hop; the f32-rec
                        # mul at 1x is cheaper than waiting on ACT
                        rec_h = rec
                    o = outp.tile([P, CT * NCH], BF, tag="o", name="o")
                    grp = 2
                    for cp in range(CT // grp):
                        rec_b = bass.AP(tensor=rec_h.tensor, offset=rec_h.offset,
                                        ap=[rec_h.ap[0], [0, grp], rec_h.ap[1]])
                        sl = slice(grp * cp, grp * cp + grp)
                        osl = o.rearrange("p (c n) -> p c n", c=CT)[:, sl, :]
                        usl = u_src.rearrange("p (c n) -> p c n", c=CT)[:, sl, :]
                        xsl = xr_r[:, sl, ch * NCH:(ch + 1) * NCH]
                        nc.vector.tensor_mul(osl, usl, rec_b)
                        nc.vector.tensor_add(osl, osl, xsl)
                        dq = nc.scalar if (last and cp % 2 == 1) else nc.sync
                        dq.dma_start(
                            out=out_r[:, sl, ch * NCH:(ch + 1) * NCH],
                            in_=osl)
    nc.finalize()
    return nc


def _get_program():
    if "nc" not in _CACHE:
        _CACHE["nc"] = _build_program()
    return _CACHE["nc"]


def _f8(a):
    return np.clip(np.asarray(a, np.float32), -240.0, 240.0).astype(F8E4NP)


def _pairs(w):
    # w: [C, O] channel-major weight -> [128, TP, J, O] DoubleRow pair layout
    o = w.shape[1]
    return np.ascontiguousarray(
        w.reshape(TP, 2, P, o).transpose(2, 0, 1, 3).reshape(P, TP * 2 * o))


def make_in_maps(x, q_w, q_b, k_w, k_b, v_w, v_b, gamma):
    x = np.asarray(x, dtype=np.float32)
    gamma_f = float(np.asarray(gamma).reshape(-1)[0])
    qw8 = _f8(_pairs(np.asarray(q_w, np.float32).T * SQ))
    kw8 = _f8(_pairs(np.asarray(k_w, np.float32).T * SQ))
    # pad q/k weights to 128 output columns with zeros
    def pad_o(w8):
        w4 = w8.reshape(P, TP * 2, CQ)
        z = np.zeros((P, TP * 2, P - CQ), F8E4NP)
        return np.ascontiguousarray(
            np.concatenate([w4, z], axis=2).reshape(P, TP * 2 * P))
    qw8 = pad_o(qw8)
    kw8 = pad_o(kw8)
    vw8 = _f8(_pairs(np.asarray(v_w, np.float32).T * (gamma_f * SV)))
    qb_c = np.zeros((P, 1), np.float32)
    qb_c[:CQ, 0] = np.asarray(q_b, np.float32)
    kb_c = np.zeros((P, 1), np.float32)
    kb_c[:CQ, 0] = np.asarray(k_b, np.float32)
    gvb = (gamma_f * np.asarray(v_b, np.float32)).reshape(C, 1)

    xf = x.reshape(B, C, N)
    in_maps = []
    for core in range(NCORES):
        b, h = core // 2, core % 2
        mine = xf[b, :, h * NQ:(h + 1) * NQ]
        other = xf[b, :, (1 - h) * NQ:(2 - h) * NQ]
        x_perm = np.concatenate([mine, other], axis=1)
        in_maps.append({
            "x8": np.ascontiguousarray(
                _f8(x_perm).reshape(TP, 2, P, N).transpose(2, 0, 1, 3)
                .reshape(P, TP * 2 * N)),
            "xr": (mine + gvb).astype(BF16),
            "qw": qw8, "kw": kw8, "vw": vw8,
            "qb": qb_c, "kb": kb_c,
        })
    return in_maps


def run(in_maps, **kwargs):
    nc = _get_program()
    return run_bass_kernel_spmd(nc, in_maps, list(range(NCORES)), **kwargs)


def gather(results):
    out = np.empty((B, C, N), dtype=np.float32)
    for core in range(NCORES):
        b, h = core // 2, core % 2
        out[b, :, h * NQ:(h + 1) * NQ] = np.asarray(
            results[core]["out"], dtype=np.float32)
    return out.reshape(B, C, H, W)


def kernel(x, q_w, q_b, k_w, k_b, v_w, v_b, gamma, **_):
    in_maps = make_in_maps(x, q_w, q_b, k_w, k_b, v_w, v_b, gamma)
    res = run(in_maps)
    return gather(res.results)


# revision 59
# speedup vs baseline: 1.0286x; 1.0286x over previous
"""Position-attention kernel for Trainium2 (8 NeuronCores, Bass/Tile).

Module: q,k = 1x1 convs to C/8 channels, v = 1x1 conv, attn = softmax(q^T k),
y = v @ attn^T, out = gamma*y + x.  Shapes: B=4, C=512, H=W=64 (N=4096, Cq=64).

Sharding: data-parallel over batch x query-halves -> 8 cores. Core i handles
batch i//2, query positions [h*2048, (h+1)*2048) with h = i%2.

v2 design (all-fp8 projections, wide exps, detached epilogue):
- x ships once as fp8e4 in DoubleRow pair layout; K/Q/V projections all run as
  fp8 DR matmuls in the same (128,128) PE tiling mode as the attention loop, so
  the PE never mode-switches. q/k weights are scaled x16 (v: x32*gamma) on host
  to clear the fp8e4 denormal range; the PSUM-drain activations descale.
- K/Q weights are zero-padded to 128 output columns, so the k/q SBUF tiles come
  out zero-padded to 128 partitions with no memsets.
- Main loop per t (2 key tiles): 2 bf16 ST matmuls into a 2-bank PSUM pair, ONE
  1024-wide exp -> fp8e5, a bf16 running denominator add on DVE, 4 fp8-DR U
  matmuls. U matmuls are emitted one t behind the STs to keep the PE queue from
  stalling on ACT.
- Per-chunk epilogue: u is drained PSUM->SBUF as bf16 on DVE (frees the PSUM
  banks fast), denominator partition-reduce on GpSimd (idle engine, hidden
  latency), reciprocal + normalize + residual on DVE, bf16 output DMA. The last
  chunk reduces via an all-ones PE matmul instead to shorten the tail.
- Residual ships as bf16 with gamma*v_b pre-added; output DMA is bf16 and the
  host upcasts.
"""

import numpy as np
import ml_dtypes

import concourse.bass as bass
import concourse.mybir as mybir
import concourse.tile as tile
from concourse import bacc, bass_isa
from concourse.bass_utils import run_bass_kernel_spmd

BF16 = ml_dtypes.bfloat16
F8E4NP = ml_dtypes.float8_e4m3

B, C, H, W = 4, 512, 64, 64
N = H * W            # 4096 keys per batch
NQ = N // 2          # 2048 queries per core
CQ = C // 8          # 64 q/k channels
P = 128
CT = C // P          # 4 channel tiles
TP = 2               # channel-tile pairs (DoubleRow)
MT = N // P          # 32 key tiles
NCH = 512            # matmul moving-dim chunk
QCH = NQ // NCH      # 4 query chunks per core
KCH = N // NCH       # 8 key chunks
NCORES = 8

F32 = mybir.dt.float32
BF = mybir.dt.bfloat16
F8 = mybir.dt.float8e4
F8E = mybir.dt.float8e5
AF = mybir.ActivationFunctionType
DR = mybir.MatmulPerfMode.DoubleRow
LN16 = 2.772588722239781  # exp shift (ln 16): max logit ~10.9 -> e^8.1 < fp8e5 max
SQ = 16.0            # host scale on q/k weights (fp8e4 denormal avoidance)
SV = 32.0            # host scale on gamma*v weights

_CACHE = {}


def _build_program():
    nc = bacc.Bacc()

    x8 = nc.declare_dram_parameter("x8", [P, TP * TP * N], F8, isOutput=False)
    xr = nc.declare_dram_parameter("xr", [C, NQ], BF, isOutput=False)
    qw = nc.declare_dram_parameter("qw", [P, TP * TP * P], F8, isOutput=False)
    kw = nc.declare_dram_parameter("kw", [P, TP * TP * P], F8, isOutput=False)
    vw = nc.declare_dram_parameter("vw", [P, TP * TP * C], F8, isOutput=False)
    qb = nc.declare_dram_parameter("qb", [P, 1], F32, isOutput=False)
    kb = nc.declare_dram_parameter("kb", [P, 1], F32, isOutput=False)
    out = nc.declare_dram_parameter("out", [C, NQ], BF, isOutput=True)

    with tile.TileContext(nc) as tc:
        with tc.tile_pool(name="consts", bufs=1) as consts:
            x8_sb = consts.tile([P, TP * TP * N], F8)
            qw_sb = consts.tile([P, TP * TP * P], F8)
            kw_sb = consts.tile([P, TP * TP * P], F8)
            vw_sb = consts.tile([P, TP * TP * C], F8)
            qb_sb = consts.tile([P, 1], F32)
            kb_sb = consts.tile([P, 1], F32)
            xr_sb = consts.tile([P, CT * NQ], BF)
            k_sb = consts.tile([P, N], BF)
            q_sb = consts.tile([P, NQ], BF)
            vt_sb = consts.tile([P, MT * C], F8)       # vT: 32 m-tiles of [128, 512]
            ln16_sb = consts.tile([P, 1], F32)
            ones_sb = consts.tile([P, P], BF)
            ones8_sb = consts.tile([P, P], F8)

            # ---- input DMAs ----
            # first x8 slab + small weights first so the K projection can
            # start as early as possible; residual last (not needed until the
            # first epilogue).
            x8_r = x8[:, :].rearrange("p (t n) -> p t n", t=TP * TP)
            x8sb_r = x8_sb.rearrange("p (t n) -> p t n", t=TP * TP)

            def x8_slab(q, s):
                q.dma_start(out=x8sb_r[:, :, s * NCH:(s + 1) * NCH],
                            in_=x8_r[:, :, s * NCH:(s + 1) * NCH])

            nc.scalar.dma_start(out=kw_sb, in_=kw[:, :])
            x8_slab(nc.sync, 0)
            nc.scalar.dma_start(out=qw_sb, in_=qw[:, :])
            nc.scalar.dma_start(out=kb_sb, in_=kb[:, :])
            nc.scalar.dma_start(out=qb_sb, in_=qb[:, :])
            nc.sync.dma_start(out=vw_sb, in_=vw[:, :])
            for s in range(1, KCH):
                x8_slab(nc.sync if s % 2 == 0 else nc.scalar, s)
            nc.scalar.dma_start(out=xr_sb.rearrange("p (t m) -> p t m", t=CT),
                                in_=xr[:, :].rearrange("(t p) m -> p t m", p=P))

            nc.vector.memset(ln16_sb, -LN16)
            nc.vector.memset(ones_sb, 1.0)
            nc.vector.memset(ones8_sb, 1.0)
            # Touch bias tiles on ACT (single sync-wait slot on the
            # activation-with-bias struct) + preload the Exp table set early.
            bias_touch = consts.tile([P, 2], F32)
            nc.scalar.activation(bias_touch[:, 0:1], kb_sb, AF.Copy)
            nc.scalar.activation(bias_touch[:, 1:2], qb_sb, AF.Copy)
            exp_touch = consts.tile([P, 1], F32)
            nc.scalar.activation(exp_touch, ln16_sb, AF.Exp, bias=ln16_sb)

            kw_r = kw_sb.rearrange("p (t j o) -> p t j o", t=TP, j=TP)
            qw_r = qw_sb.rearrange("p (t j o) -> p t j o", t=TP, j=TP)
            vw_r = vw_sb.rearrange("p (t j o) -> p t j o", t=TP, j=TP)
            x8_4 = x8_sb.rearrange("p (t j n) -> p t j n", t=TP, j=TP)

            # ---- K/Q projections (all fp8 DoubleRow) ----
            # V projections are NOT emitted here: they are interleaved into
            # chunk 0 of the attention loop below (their PSUM quads share the
            # st pool; quad t produces exactly the vt pair iteration t needs),
            # so their PE work fills dependency-wait slack and their descale
            # copies hide under the exps.
            with tc.tile_pool(name="proj_kq", bufs=2, space="PSUM") as proj_kq:
                def kq_drain(dst, src, bias, alt):
                    # descale + bias: alternate ACT / DVE so neither engine's
                    # PSUM-ring drain gates the projection matmul stream
                    if alt:
                        nc.vector.tensor_scalar(dst, src, 1.0 / SQ, bias,
                                                mybir.AluOpType.mult,
                                                mybir.AluOpType.add)
                    else:
                        nc.scalar.activation(dst, src, AF.Identity, bias=bias,
                                             scale=1.0 / SQ)

                def k_proj(ch):
                    kp = proj_kq.tile([P, NCH], F32, tag="kq", name="kp")
                    for tp in range(TP):
                        nc.tensor.matmul(
                            kp, lhsT=kw_r[:, tp, :, :],
                            rhs=x8_4[:, tp, :, ch * NCH:(ch + 1) * NCH],
                            start=(tp == 0), stop=(tp == TP - 1), perf_mode=DR)
                    kq_drain(k_sb[:, ch * NCH:(ch + 1) * NCH], kp, kb_sb,
                             ch % 2 == 1)

                def q_proj(ch):
                    qp = proj_kq.tile([P, NCH], F32, tag="kq", name="qp")
                    for tp in range(TP):
                        nc.tensor.matmul(
                            qp, lhsT=qw_r[:, tp, :, :],
                            rhs=x8_4[:, tp, :, ch * NCH:(ch + 1) * NCH],
                            start=(tp == 0), stop=(tp == TP - 1), perf_mode=DR)
                    kq_drain(q_sb[:, ch * NCH:(ch + 1) * NCH], qp, qb_sb,
                             ch % 2 == 0)

                for s in range(KCH):
                    k_proj(s)
                    if s < QCH:
                        q_proj(s)

            # ---- attention main loop ----
            vt_r = vt_sb.rearrange("p (m c) -> p m c", m=MT)
            out_r = out[:, :].rearrange("(c p) n -> p c n", p=P)
            xr_r = xr_sb.rearrange("p (c m) -> p c m", c=CT)
            with (
                tc.tile_pool(name="u_ps", bufs=1, space="PSUM") as u_ps,
                tc.tile_pool(name="st_ps", bufs=2, space="PSUM") as st_ps,
                tc.tile_pool(name="e_pool", bufs=8) as e_pool,
                tc.tile_pool(name="cs_pool", bufs=2) as cs_pool,
                tc.tile_pool(name="fin", bufs=2) as fin,
                tc.tile_pool(name="outp", bufs=2) as outp,
            ):
                def v_proj(mt2):
                    # two key tiles (2*mt2, 2*mt2+1) -> one [128, 1024] PSUM
                    # quad (borrowed from the st ring) -> one wide descaling
                    # copy to fp8 vt.
                    vp = st_ps.tile([P, 2 * NCH], F32, tag="st", name="vp")
                    for half in range(2):
                        mt = 2 * mt2 + half
                        for tp in range(TP):
                            nc.tensor.matmul(
                                vp[:, half * NCH:(half + 1) * NCH],
                                lhsT=x8_4[:, tp, :, mt * P:(mt + 1) * P],
                                rhs=vw_r[:, tp, :, :],
                                start=(tp == 0), stop=(tp == TP - 1),
                                perf_mode=DR)
                    dst = vt_sb[:, 2 * mt2 * C:(2 * mt2 + 2) * C]
                    # alternate the f32->fp8 descale copies between ACT and
                    # DVE; DVE takes the EVEN quads so its copy never lands on
                    # the same iteration as the cs pair-add (odd t)
                    if mt2 % 2 == 1:
                        nc.scalar.activation(dst, vp, AF.Copy, scale=1.0 / SV)
                    else:
                        nc.vector.tensor_scalar_mul(dst, vp, 1.0 / SV)

                for ch in range(QCH):
                    last = ch == QCH - 1
                    u = u_ps.tile([P, CT * NCH], F32, tag="u", name="u")
                    cs_d = cs_pool.tile([P, 4 * NCH], BF, tag="csd", name="cs_d")
                    cs_n = cs_d[:, :2 * NCH]   # last chunk: fine-grained accum
                    qs = q_sb[:, ch * NCH:(ch + 1) * NCH]

                    pend = []   # deferred U matmuls (one t behind)

                    def flush_u():
                        while pend:
                            pend.pop(0)()

                    ep = None
                    for t in range(MT // 2):
                        if ch == 0:
                            # interleave the V projection: quad t produces the
                            # vt pair that this iteration's (deferred) U uses
                            v_proj(t)
                        stp = st_ps.tile([P, 2 * NCH], F32, tag="st", name="st")
                        for j in range(2):
                            nc.tensor.matmul(
                                stp[:, j * NCH:(j + 1) * NCH],
                                lhsT=k_sb[:, (2 * t + j) * P:(2 * t + j + 1) * P],
                                rhs=qs, start=True, stop=True)
                        if t % 2 == 0:
                            ep = e_pool.tile([P, 2, 2 * NCH], F8E, tag="e", name="ep")
                        half = t % 2
                        e1k = ep[:, half, :]
                        nc.scalar.activation(e1k, stp, AF.Exp, bias=ln16_sb)
                        flush_u()

                        def emit_u(t=t, e2=e1k.rearrange("p (j n) -> p j n", j=2)):
                            for c in range(CT):
                                nc.tensor.matmul(
                                    u[:, c * NCH:(c + 1) * NCH],
                                    lhsT=vt_r[:, 2 * t:2 * t + 2, c * P:(c + 1) * P],
                                    rhs=e2,
                                    start=(t == 0), stop=(t == MT // 2 - 1),
                                    perf_mode=DR)
                        pend.append(emit_u)

                        # the final pair (t=14,15) is never accumulated on
                        # DVE: the epilogue's all-ones PE matmuls read its
                        # exp tile directly in fp8 (keeps DVE under the
                        # per-chunk wall so rec never drifts late)
                        if t >= MT // 2 - 2:
                            if t == MT // 2 - 1:
                                ep7 = ep
                        elif last:
                            # fine-grained per-t adds (tail latency)
                            if t == 0:
                                nc.vector.tensor_copy(cs_n, e1k)
                            else:
                                nc.vector.tensor_add(cs_n, cs_n, e1k)
                        elif half == 1:
                            # accumulate the finished pair tile (2048 wide)
                            pt = t // 2
                            epf = ep.rearrange("p j n -> p (j n)")
                            if pt == 0:
                                nc.vector.tensor_copy(cs_d, epf)
                            else:
                                nc.vector.tensor_add(cs_d, cs_d, epf)
                    flush_u()

                    # ---- epilogue ----
                    if not last:
                        # free the U PSUM banks quickly: bf16 drain split
                        # ACT/DVE so the boundary exps only wait one ACT op
                        uc = outp.tile([P, CT * NCH], BF, tag="uc", name="uc")
                        nc.scalar.activation(uc[:, :2 * NCH], u[:, :2 * NCH], AF.Copy)
                        nc.vector.tensor_copy(uc[:, 2 * NCH:], u[:, 2 * NCH:])
                        u_src = uc
                    else:
                        u_src = u
                    # fold + partition-reduce + broadcast via accumulating
                    # all-ones PE matmuls. Non-last chunks write into the u
                    # c0 bank (just freed by the drain, so the st ring isn't
                    # stolen from the next chunk's STs); the last chunk uses
                    # a free st bank since its u is still being read.
                    # the reduce matmuls write into a free st bank: both st
                    # buffers are idle at chunk end, and with the cs chain
                    # finishing early the DVE reciprocal frees it quickly —
                    # unlike the u bank, this does not chain the next chunk's
                    # STs behind the ACT drain in PE's in-order queue
                    rec = fin.tile([P, NCH], F32, tag="rec", name="rec")
                    csp = st_ps.tile([P, 2 * NCH], F32, tag="st",
                                     name="csp")[:, :NCH]
                    if not last:
                        csrc, nblk = cs_d, 4
                    else:
                        csrc, nblk = cs_n, 2
                    # ep7 blocks first: they only depend on the last two
                    # exps, so accumulation starts while the trailing cs
                    # add finishes
                    ep7f = ep7.rearrange("p h n -> p (h n)")
                    for b in range(4):
                        nc.tensor.matmul(csp, lhsT=ones8_sb,
                                         rhs=ep7f[:, b * NCH:(b + 1) * NCH],
                                         start=(b == 0), stop=False)
                    for b in range(nblk):
                        nc.tensor.matmul(csp, lhsT=ones_sb,
                                         rhs=csrc[:, b * NCH:(b + 1) * NCH],
                                         start=False, stop=(b == nblk - 1))
                    nc.vector.reciprocal_approx_fast(out=rec, in_=csp)
                    if not last:
                        # bf16 rec (on ACT) lets the normalize muls hit DVE's
                        # 2x 16-bit mode
                        rec_h = fin.tile([P, NCH], BF, tag="rech", name="rec_h")
                        nc.scalar.activation(rec_h, rec, AF.Copy)
                    else:
                        # tail: skip the cross-engine cast hop; the f32-rec
                        # mul at 1x is cheaper than waiting on ACT
                        rec_h = rec
                    o = outp.tile([P, CT * NCH], BF, tag="o", name="o")
                    grp = 2
                    for cp in range(CT // grp):
                        rec_b = bass.AP(tensor=rec_h.tensor, offset=rec_h.offset,
                                        ap=[rec_h.ap[0], [0, grp], rec_h.ap[1]])
                        sl = slice(grp * cp, grp * cp + grp)
                        osl = o.rearrange("p (c n) -> p c n", c=CT)[:, sl, :]
                        usl = u_src.rearrange("p (c n) -> p c n", c=CT)[:, sl, :]
                        xsl = xr_r[:, sl, ch * NCH:(ch + 1) * NCH]
                        nc.vector.tensor_mul(osl, usl, rec_b)
                        nc.vector.tensor_add(osl, osl, xsl)
                        dq = nc.scalar if (last and cp % 2 == 1) else nc.sync
                        dq.dma_start(
                            out=out_r[:, sl, ch * NCH:(ch + 1) * NCH],
                            in_=osl)
    nc.finalize()
    return nc


def _get_program():
    if "nc" not in _CACHE:
        _CACHE["nc"] = _build_program()
    return _CACHE["nc"]


def _f8(a):
    return np.clip(np.asarray(a, np.float32), -240.0, 240.0).astype(F8E4NP)


def _pairs(w):
    # w: [C, O] channel-major weight -> [128, TP, J, O] DoubleRow pair layout
    o = w.shape[1]
    return np.ascontiguousarray(
        w.reshape(TP, 2, P, o).transpose(2, 0, 1, 3).reshape(P, TP * 2 * o))


def make_in_maps(x, q_w, q_b, k_w, k_b, v_w, v_b, gamma):
    x = np.asarray(x, dtype=np.float32)
    gamma_f = float(np.asarray(gamma).reshape(-1)[0])
    qw8 = _f8(_pairs(np.asarray(q_w, np.float32).T * SQ))
    kw8 = _f8(_pairs(np.asarray(k_w, np.float32).T * SQ))
    # pad q/k weights to 128 output columns with zeros
    def pad_o(w8):
        w4 = w8.reshape(P, TP * 2, CQ)
        z = np.zeros((P, TP * 2, P - CQ), F8E4NP)
        return np.ascontiguousarray(
            np.concatenate([w4, z], axis=2).reshape(P, TP * 2 * P))
    qw8 = pad_o(qw8)
    kw8 = pad_o(kw8)
    vw8 = _f8(_pairs(np.asarray(v_w, np.float32).T * (gamma_f * SV)))
    qb_c = np.zeros((P, 1), np.float32)
    qb_c[:CQ, 0] = np.asarray(q_b, np.float32)
    kb_c = np.zeros((P, 1), np.float32)
    kb_c[:CQ, 0] = np.asarray(k_b, np.float32)
    gvb = (gamma_f * np.asarray(v_b, np.float32)).reshape(C, 1)

    xf = x.reshape(B, C, N)
    in_maps = []
    for core in range(NCORES):
        b, h = core // 2, core % 2
        mine = xf[b, :, h * NQ:(h + 1) * NQ]
        other = xf[b, :, (1 - h) * NQ:(2 - h) * NQ]
        x_perm = np.concatenate([mine, other], axis=1)
        in_maps.append({
            "x8": np.ascontiguousarray(
                _f8(x_perm).reshape(TP, 2, P, N).transpose(2, 0, 1, 3)
                .reshape(P, TP * 2 * N)),
            "xr": (mine + gvb).astype(BF16),
            "qw": qw8, "kw": kw8, "vw": vw8,
            "qb": qb_c, "kb": kb_c,
        })
    return in_maps


def run(in_maps, **kwargs):
    nc = _get_program()
    return run_bass_kernel_spmd(nc, in_maps, list(range(NCORES)), **kwargs)


def gather(results):
    out = np.empty((B, C, N), dtype=np.float32)
    for core in range(NCORES):
        b, h = core // 2, core % 2
        out[b, :, h * NQ:(h + 1) * NQ] = np.asarray(
            results[core]["out"], dtype=np.float32)
    return out.reshape(B, C, H, W)


def kernel(x, q_w, q_b, k_w, k_b, v_w, v_b, gamma, **_):
    in_maps = make_in_maps(x, q_w, q_b, k_w, k_b, v_w, v_b, gamma)
    res = run(in_maps)
    return gather(res.results)
